# revision 3
# baseline (speedup 1.0000x reference)
"""Contrastive loss (SimCLR-style) on 8 TRN2 NeuronCores.

loss = -mean(diag(log_softmax(zi_n @ zj_n^T / T)))  with zi_n, zj_n L2-normalized,
N=4096, D=256, T=0.5.

Sharding: 16 quarter-block cover. Core pair (2q, 2q+1) owns zi quarter q
(1024 rows). Core 2q gets zj cols = quarters {q, q+1}; core 2q+1 gets
{q+2, q+3} (mod 4). Every (row-quarter, col-quarter) block is computed
exactly once; every core's zjA input aligns row-for-row with its zi input,
so the diagonal block is always in zjA for even cores. 3MB HBM per core.

Per core: l2-normalize zj (bf16), transpose via xbar, matmul bf16
(k-accum over 2 halves), fused exp+row-sum on ScalarE with per-partition
scale 2/||zi||. Device outputs per-row partial exp-sums + diag logits;
host adds core-pair partials, takes ln, subtracts diag, means.
"""

import numpy as np

import concourse.bass as bass
import concourse.bacc as bacc
import concourse.tile as tile
import concourse.bass_utils as bass_utils
from concourse import mybir

N = 4096
D = 256
NCORES = 8
NQ = N // 4          # 1024 rows per quarter (zi rows per core)
P = 128
CH = NQ // P         # 8 rows per partition / chunks
KH = D // P          # 2 contraction halves
MAGIC = 0x5F3759DF

F32 = mybir.dt.float32
U32 = mybir.dt.uint32
BF16 = mybir.dt.bfloat16
AF = mybir.ActivationFunctionType
ALU = mybir.AluOpType
AX = mybir.AxisListType


def build_nc():
    nc = bacc.Bacc(
        "TRN2",
        target_bir_lowering=False,
        debug=False,
        enable_asserts=False,
    )
    z_i = nc.dram_tensor("z_i", (NQ, D), F32, kind="ExternalInput").ap()
    z_ja = nc.dram_tensor("z_ja", (NQ, D), F32, kind="ExternalInput").ap()
    z_jb = nc.dram_tensor("z_jb", (NQ, D), F32, kind="ExternalInput").ap()
    out = nc.dram_tensor("out", (P, 16), F32, kind="ExternalOutput").ap()

    with tile.TileContext(nc) as tc:
        with (
            tc.tile_pool(name="const", bufs=1) as const,
            tc.tile_pool(name="big", bufs=1) as big,
            tc.tile_pool(name="work", bufs=2) as work,
            tc.tile_pool(name="stat", bufs=1) as stat,
            tc.tile_pool(name="psum", bufs=4, space="PSUM") as psum,
        ):
            # pin the exp ACT table set at t=0
            dummy = const.tile([1, 1], F32)
            nc.vector.memset(dummy, 1.0)
            nc.scalar.activation(out=dummy, in_=dummy, func=AF.Exp)

            magic = const.tile([P, 16], U32)
            nc.vector.memset(magic, MAGIC)

            def rsqrt_dve(a, y, w):
                """y[:,:w] = 1/sqrt(a[:,:w]): quake seed + 1 Newton step."""
                au = a.bitcast(U32)
                yu = y.bitcast(U32)
                sh = work.tile([P, 16], U32, tag="rsq_sh")
                nc.vector.tensor_scalar(
                    out=sh[:, :w], in0=au, scalar1=1, scalar2=None,
                    op0=ALU.logical_shift_right,
                )
                nc.vector.tensor_sub(out=yu, in0=magic[:, :w], in1=sh[:, :w])
                t1 = work.tile([P, 16], F32, tag="rsq_t1")
                nc.vector.tensor_mul(out=t1[:, :w], in0=y, in1=y)
                nc.vector.tensor_mul(out=t1[:, :w], in0=t1[:, :w], in1=a)
                nc.vector.tensor_scalar(
                    out=t1[:, :w], in0=t1[:, :w], scalar1=-0.5, scalar2=1.5,
                    op0=ALU.mult, op1=ALU.add,
                )
                nc.vector.tensor_mul(out=y, in0=y, in1=t1[:, :w])

            # ---- loads: partition-major packing -> 8KB/partition descriptors
            zi_f = big.tile([P, CH, D], F32)
            nc.sync.dma_start(
                out=zi_f, in_=z_i.rearrange("(p c) d -> p c d", p=P)
            )
            zja_f = big.tile([P, CH, D], F32)
            nc.scalar.dma_start(
                out=zja_f, in_=z_ja.rearrange("(p c) d -> p c d", p=P)
            )
            zjb_f = big.tile([P, CH, D], F32)
            nc.sync.dma_start(
                out=zjb_f, in_=z_jb.rearrange("(p c) d -> p c d", p=P)
            )

            # ---- zi: cast, transpose (unscaled; 2/||zi|| applied via ACT scale)
            zi_bf = big.tile([P, CH, D], BF16)
            nc.vector.tensor_copy(out=zi_bf, in_=zi_f)
            ziT = big.tile([P, CH * KH, P], BF16)
            nc.sync.dma_start_transpose(
                out=ziT, in_=zi_bf.rearrange("p c d -> p (c d)")
            )
            nrm_i = stat.tile([P, CH], F32)
            for c in range(CH):
                sq = work.tile([P, D], BF16, tag="sq")
                nc.vector.scalar_tensor_tensor(
                    out=sq, in0=zi_bf[:, c, :], scalar=1.0, in1=zi_bf[:, c, :],
                    op0=ALU.mult, op1=ALU.mult,
                    accum_out=nrm_i[:, c : c + 1],
                )

            # ---- zjA: cast, norms, rsqrt (joint with zi), scale, transpose
            zja_bf = big.tile([P, CH, D], BF16)
            nc.vector.tensor_copy(out=zja_bf, in_=zja_f)
            nrm_ia = stat.tile([P, 16], F32)  # cols 0-7: zi, 8-15: zjA
            nc.vector.tensor_copy(out=nrm_ia[:, :CH], in_=nrm_i)
            for c in range(CH):
                sq = work.tile([P, D], BF16, tag="sq")
                nc.vector.scalar_tensor_tensor(
                    out=sq, in0=zja_bf[:, c, :], scalar=1.0, in1=zja_bf[:, c, :],
                    op0=ALU.mult, op1=ALU.mult,
                    accum_out=nrm_ia[:, CH + c : CH + c + 1],
                )
            rsq_ia = stat.tile([P, 16], F32)
            rsqrt_dve(nrm_ia, rsq_ia, 16)
            s2 = stat.tile([P, CH], F32)
            nc.vector.tensor_scalar(
                out=s2, in0=rsq_ia[:, :CH], scalar1=2.0, scalar2=None, op0=ALU.mult
            )
            zjas = big.tile([P, CH, D], BF16)
            for c in range(CH):
                nc.vector.tensor_scalar_mul(
                    out=zjas[:, c, :], in0=zja_f[:, c, :],
                    scalar1=rsq_ia[:, CH + c : CH + c + 1],
                )
            zjaT = big.tile([P, CH * KH, P], BF16)
            nc.sync.dma_start_transpose(
                out=zjaT, in_=zjas.rearrange("p c d -> p (c d)")
            )
            zjaT_r = zjaT.rearrange("do (c h) m -> do c h m", h=KH)

            # ---- diag (gpsimd; only meaningful on even cores): zi_bf . zjas
            dt = stat.tile([P, CH], F32)
            for c in range(CH):
                sqd = work.tile([P, D], BF16, tag="sqd")
                nc.vector.scalar_tensor_tensor(
                    out=sqd, in0=zi_bf[:, c, :], scalar=1.0, in1=zjas[:, c, :],
                    op0=ALU.mult, op1=ALU.mult,
                    accum_out=dt[:, c : c + 1],
                )
            dg = stat.tile([P, CH], F32)
            nc.vector.tensor_mul(out=dg, in0=dt, in1=s2)

            # ---- zjB: same pipeline
            zjb_bf = big.tile([P, CH, D], BF16)
            nc.vector.tensor_copy(out=zjb_bf, in_=zjb_f)
            nrm_b = stat.tile([P, CH], F32)
            for c in range(CH):
                sq = work.tile([P, D], BF16, tag="sq")
                nc.vector.scalar_tensor_tensor(
                    out=sq, in0=zjb_bf[:, c, :], scalar=1.0, in1=zjb_bf[:, c, :],
                    op0=ALU.mult, op1=ALU.mult,
                    accum_out=nrm_b[:, c : c + 1],
                )
            rsq_b = stat.tile([P, CH], F32)
            rsqrt_dve(nrm_b, rsq_b, CH)
            zjbs = big.tile([P, CH, D], BF16)
            for c in range(CH):
                nc.vector.tensor_scalar_mul(
                    out=zjbs[:, c, :], in0=zjb_f[:, c, :],
                    scalar1=rsq_b[:, c : c + 1],
                )
            zjbT = big.tile([P, CH * KH, P], BF16)
            nc.scalar.dma_start_transpose(
                out=zjbT, in_=zjbs.rearrange("p c d -> p (c d)")
            )
            zjbT_r = zjbT.rearrange("do (c h) m -> do c h m", h=KH)

            # ---- main compute: per n-chunk, A-half then B-half columns.
            # A-half instrs can fire while zjB still preprocesses.
            rs = stat.tile([P, 2 * CH], F32)  # cols 0-7 A-half, 8-15 B-half

            def half_tile(c, zT_r, g):
                pt = psum.tile([P, 1024], F32, tag="pt")
                for h in range(KH):
                    for jj in range(2):
                        nc.tensor.matmul(
                            pt[:, jj * 512 : (jj + 1) * 512],
                            lhsT=ziT.rearrange(
                                "do (c h) m -> do c h m", h=KH
                            )[:, c, h, :],
                            rhs=zT_r[:, jj * 4 : jj * 4 + 4, h, :],
                            start=(h == 0),
                            stop=(h == KH - 1),
                        )
                nc.scalar.activation(
                    out=pt, in_=pt, func=AF.Exp,
                    scale=s2[:, c : c + 1],
                    accum_out=rs[:, g * CH + c : g * CH + c + 1],
                )

            for c in range(CH):
                half_tile(c, zjaT_r, 0)
            for c in range(CH):
                half_tile(c, zjbT_r, 1)

            # ---- combine + output: rs_total = rsA + rsB ; out = [rs | dg]
            osb = stat.tile([P, 16], F32)
            nc.vector.tensor_add(
                out=osb[:, :CH], in0=rs[:, :CH], in1=rs[:, CH:]
            )
            nc.vector.tensor_copy(out=osb[:, CH:], in_=dg)
            nc.scalar.dma_start(out=out, in_=osb)

    nc.compile()
    return nc


_NC = None


def _get_nc():
    global _NC
    if _NC is None:
        _NC = build_nc()
    return _NC


def make_in_maps(z_i, z_j):
    """Per-core inputs for the quarter-block cover."""
    Q = [np.ascontiguousarray(z_j[q * NQ : (q + 1) * NQ], np.float32)
         for q in range(4)]
    in_maps = []
    for q in range(4):
        zi_q = np.ascontiguousarray(z_i[q * NQ : (q + 1) * NQ], np.float32)
        in_maps.append(
            {"z_i": zi_q, "z_ja": Q[q], "z_jb": Q[(q + 1) % 4]}
        )
        in_maps.append(
            {"z_i": zi_q, "z_ja": Q[(q + 2) % 4], "z_jb": Q[(q + 3) % 4]}
        )
    return in_maps


def combine(results):
    """Host: add core-pair exp-sums, ln, subtract diag, mean."""
    total = 0.0
    for q in range(4):
        oa = results[2 * q]["out"].astype(np.float64)
        ob = results[2 * q + 1]["out"].astype(np.float64)
        rs_total = oa[:, :CH] + ob[:, :CH]   # [128, 8]
        dgq = oa[:, CH:16]                   # diag logits (even core's zjA)
        total += float(np.sum(np.log(rs_total) - dgq))
    return np.float32(total / N)


def kernel(z_i: np.ndarray, z_j: np.ndarray, **_unused) -> np.ndarray:
    z_i = np.ascontiguousarray(z_i, dtype=np.float32)
    z_j = np.ascontiguousarray(z_j, dtype=np.float32)
    nc = _get_nc()
    res = bass_utils.run_bass_kernel_spmd(
        nc, make_in_maps(z_i, z_j), core_ids=list(range(NCORES))
    )
    return combine(res.results)
